# revision 25
# baseline (speedup 1.0000x reference)
import sys

sys.path.insert(0, "/opt/trn_rl_repo")
import numpy as np

B, DIM, H, W = 2, 192, 64, 64
HEADS = 4
C = DIM // HEADS  # 48 per-head channels
HW = H * W  # 4096
NCORES = 8
QQ = 4  # q quarters of 1024
QW = HW // QQ  # 1024
KCH = HW // 128  # 32 k-chunks

_cache = {}


def _build():
    import concourse.bass as bass
    import concourse.tile as tile
    from concourse import bacc, mybir

    F32 = mybir.dt.float32
    F16 = mybir.dt.float16
    AF = mybir.ActivationFunctionType

    nc = bacc.Bacc("TRN2", target_bir_lowering=False, debug=False,
                   num_devices=NCORES)
    x_a_d = nc.dram_tensor("x_a", [128, HW], F16, kind="ExternalInput").ap()
    x_b_d = nc.dram_tensor("x_b", [64, HW], F16, kind="ExternalInput").ap()
    w1qk_a_d = nc.dram_tensor("w1qk_a", [128, 112], F16,
                              kind="ExternalInput").ap()
    w1qk_b_d = nc.dram_tensor("w1qk_b", [64, 112], F16,
                              kind="ExternalInput").ap()
    w1v_a_d = nc.dram_tensor("w1v_a", [128, 48], F16,
                             kind="ExternalInput").ap()
    w1v_b_d = nc.dram_tensor("w1v_b", [64, 48], F16,
                             kind="ExternalInput").ap()
    dwqk_d = nc.dram_tensor("dwqk", [112, 9 * 112], F16,
                            kind="ExternalInput").ap()
    dwv_d = nc.dram_tensor("dwv", [48, 9 * 48], F16,
                           kind="ExternalInput").ap()
    pw2a_d = nc.dram_tensor("pw2a", [128, 128], F16,
                            kind="ExternalInput").ap()
    pw2b_d = nc.dram_tensor("pw2b", [128, 64], F16,
                            kind="ExternalInput").ap()
    ones2_d = nc.dram_tensor("ones2", [112, 2], F16,
                             kind="ExternalInput").ap()
    idh_d = nc.dram_tensor("identh", [128, 128], F16,
                           kind="ExternalInput").ap()
    tsc_d = nc.dram_tensor("tscaleh", [64, 1], F32,
                           kind="ExternalInput").ap()
    zmask_d = nc.dram_tensor("zmask", [128, 1], F16,
                             kind="ExternalInput").ap()
    out_d = nc.dram_tensor("out", [DIM, HW], F32, kind="ExternalOutput").ap()
    # DRAM scratch for partition<->free reshapes
    ssd = [nc.dram_tensor(f"ssd{h}", [2, HW // 2], F16, kind="Internal").ap()
           for h in range(2)]
    rqd = [nc.dram_tensor(f"rqd{h}", [2, HW // 2], F16, kind="Internal").ap()
           for h in range(2)]
    ssd_col = [t.rearrange("a (c f) -> (a c) f", c=32) for t in ssd]
    rqd_col = [t.rearrange("a (c f) -> (a c) f", c=32) for t in rqd]
    zdr = nc.dram_tensor("zdr", [1, HW], F16, kind="Internal").ap()
    rdr = nc.dram_tensor("rdr", [1, HW], F16, kind="Internal").ap()
    zdr_col = zdr.rearrange("a (j p) -> (a p) j", p=128)  # [128, 32]
    rdr_col = rdr.rearrange("a (j p) -> (a p) j", p=128)  # [128, 32]

    with tile.TileContext(nc) as tc:
        with (
            tc.tile_pool(name="persist", bufs=1) as pp,
            tc.tile_pool(name="epool", bufs=6) as ep,
            tc.tile_pool(name="chn", bufs=2) as chn,
        ):
            # ---- persistent SBUF tiles
            QHD = pp.tile([128, HW], F16, tag="QHD")  # qhat at 0:48 & 64:112
            KHb = pp.tile([128, HW], F16, tag="KHb")  # t*khat at 0:48 & 64:112
            vT = pp.tile([128, KCH * 64], F16, tag="vT")  # vt + ones col 48
            U2 = pp.tile([128, HW], F16, tag="U2")    # av accum (Z @48,@112)
            att = pp.tile([128, HW], F16, tag="att")
            identh = pp.tile([128, 128], F16, tag="identh")
            w1qk_a = pp.tile([128, 112], F16, tag="w1qk_a")
            w1qk_b = pp.tile([64, 112], F16, tag="w1qk_b")
            w1v_a = pp.tile([128, 48], F16, tag="w1v_a")
            w1v_b = pp.tile([64, 48], F16, tag="w1v_b")
            dwqk = pp.tile([112, 9 * 112], F16, tag="dwqk")
            dwv = pp.tile([48, 9 * 48], F16, tag="dwv")
            pw2a = pp.tile([128, 128], F16, tag="pw2a")
            pw2b = pp.tile([128, 64], F16, tag="pw2b")
            ones2 = pp.tile([112, 2], F16, tag="ones2")
            tscaleh = pp.tile([64, 1], F32, tag="tscaleh")
            zmask = pp.tile([128, 1], F16, tag="zmask")
            ones_row = pp.tile([1, 128], F16, tag="ones_row")
            TQK = pp.tile([112, HW], F16, tag="TQK")
            TV = pp.tile([48, HW], F16, tag="TV")
            PK = pp.tile([112, HW], F16, tag="PK")     # q' 0:48, k' 64:112
            v_sb = pp.tile([48, HW], F16, tag="v_sb")
            SQ = pp.tile([112, HW], F16, tag="SQ")
            ss_sb = pp.tile([2, HW], F16, tag="ss_sb")
            rq0 = pp.tile([1, HW], F16, tag="rq0")
            rk0 = pp.tile([1, HW], F16, tag="rk0")
            rqb = pp.tile([48, HW], F16, tag="rqb")
            rkb = pp.tile([112, HW], F16, tag="rkb")
            x_a = pp.tile([128, HW], F16, tag="x_a")
            x_b = pp.tile([64, HW], F16, tag="x_b")
            z_row = pp.tile([1, HW], F16, tag="z_row")
            rz_row = pp.tile([1, HW], F16, tag="rz_row")
            zh_row = pp.tile([1, QW], F16, tag="zh_row")
            rzh = pp.tile([128, 8], F16, tag="rzh")
            rzh_row = pp.tile([1, QW], F16, tag="rzh_row")

            nc.sync.dma_start(x_a[:], x_a_d[:])
            nc.sync.dma_start(x_b[:], x_b_d[:])
            nc.sync.dma_start(identh[:], idh_d[:])
            nc.sync.dma_start(w1qk_a[:], w1qk_a_d[:])
            nc.sync.dma_start(w1qk_b[:], w1qk_b_d[:])
            nc.sync.dma_start(w1v_a[:], w1v_a_d[:])
            nc.sync.dma_start(w1v_b[:], w1v_b_d[:])
            nc.sync.dma_start(dwqk[:], dwqk_d[:])
            nc.sync.dma_start(dwv[:], dwv_d[:])
            nc.sync.dma_start(pw2a[:], pw2a_d[:])
            nc.sync.dma_start(pw2b[:], pw2b_d[:])
            nc.sync.dma_start(ones2[:], ones2_d[:])
            nc.sync.dma_start(tscaleh[:], tsc_d[:])
            nc.sync.dma_start(zmask[:], zmask_d[:])
            nc.gpsimd.memset(ones_row[:], 1.0)
            nc.gpsimd.memset(vT[:], 0.0)
            vT3 = vT[:].rearrange("p (k m) -> p k m", m=64)
            nc.gpsimd.memset(vT3[:, :, 48:49], 1.0)

            with tc.tile_pool(name="pAB", bufs=4, space="PSUM") as pAB:
                # HAM warm-up burst: contiguous MM run so the clock-gate
                # opens before the conv phase
                for r in range(12):
                    dmy = pAB.tile([48, 512], F32, tag="B")
                    nc.tensor.matmul(
                        dmy[:], w1v_a[:],
                        x_a[:, 512 * (r % 8):512 * (r % 8) + 512],
                        start=True, stop=True, skip_group_check=True)

                # ---- phase 1: 1x1 conv, q|k block layout [112], v [48]
                for n in range(8):
                    s = slice(512 * n, 512 * (n + 1))
                    T1 = pAB.tile([112, 512], F32, tag="A")
                    nc.tensor.matmul(T1[:], w1qk_a[:], x_a[:, s],
                                     start=True, stop=False)
                    nc.tensor.matmul(T1[:], w1qk_b[:], x_b[:, s],
                                     start=False, stop=True)
                    T1v = pAB.tile([48, 512], F32, tag="B")
                    nc.tensor.matmul(T1v[:], w1v_a[:], x_a[:, s],
                                     start=True, stop=False)
                    nc.tensor.matmul(T1v[:], w1v_b[:], x_b[:, s],
                                     start=False, stop=True)
                    if n % 2 == 0:
                        nc.vector.tensor_copy(TQK[:, s], T1[:])
                        nc.scalar.copy(TV[:, s], T1v[:])
                    else:
                        nc.scalar.copy(TQK[:, s], T1[:])
                        nc.vector.tensor_copy(TV[:, s], T1v[:])

                # ---- phase 2: depthwise 3x3, block-diag q|k + v, halves
                TQK3 = TQK[:].rearrange("p (y x) -> p y x", x=64)
                TV3 = TV[:].rearrange("p (y x) -> p y x", x=64)
                taps = [(0, 0)] + [(dy, dx) for dy in (-1, 0, 1)
                                   for dx in (-1, 0, 1) if (dy, dx) != (0, 0)]
                for hh in range(2):
                    ns = range(4 * hh, 4 * hh + 4)
                    s2 = slice(2048 * hh, 2048 * (hh + 1))
                    if hh == 1:
                        for r in range(12):
                            dmy = pAB.tile([48, 512], F32, tag="B")
                            nc.tensor.matmul(
                                dmy[:], w1v_a[:],
                                x_a[:, 512 * (r % 8):512 * (r % 8) + 512],
                                start=True, stop=True, skip_group_check=True)
                    DQ = {n: pAB.tile([112, 8, 64], F32, tag="A", name=f"DQ{n}")
                          for n in ns}
                    DV = {n: pAB.tile([48, 8, 64], F32, tag="B", name=f"DV{n}")
                          for n in ns}
                    for ti, (dy, dx) in enumerate(taps):
                        t = (dy + 1) * 3 + (dx + 1)
                        first = ti == 0
                        last = ti == len(taps) - 1
                        gy0, gy1 = max(0, -dy), 64 - max(0, dy)
                        x0, x1 = max(0, -dx), 64 - max(0, dx)
                        for dst, wt, src, cw in (
                            (DQ, dwqk, TQK3, 112), (DV, dwv, TV3, 48),
                        ):
                            for n in ns:
                                sy0 = max(8 * n, gy0)
                                sy1 = min(8 * n + 8, gy1)
                                if sy1 <= sy0:
                                    continue
                                oy = slice(sy0 - 8 * n, sy1 - 8 * n)
                                ox = slice(x0, x1)
                                iy = slice(sy0 + dy, sy1 + dy)
                                ix = slice(x0 + dx, x1 + dx)
                                nc.tensor.matmul(
                                    dst[n][:, oy, ox],
                                    wt[:, cw * t:cw * t + cw],
                                    src[:, iy, ix], start=first, stop=last,
                                    skip_group_check=True)
                    for n in ns:
                        s = slice(512 * n, 512 * (n + 1))
                        DQf = DQ[n][:].rearrange("p y x -> p (y x)")
                        DVf = DV[n][:].rearrange("p y x -> p (y x)")
                        if n % 2 == 0:
                            nc.vector.tensor_copy(PK[:, s], DQf)
                            nc.scalar.copy(v_sb[:, s], DVf)
                        else:
                            nc.scalar.copy(PK[:, s], DQf)
                            nc.vector.tensor_copy(v_sb[:, s], DVf)
                        # squares + per-position sum of squares (q r0, k r1)
                        nc.vector.tensor_mul(SQ[:, s], PK[:, s], PK[:, s])
                        ssP = pAB.tile([2, 512], F32, tag="B")
                        nc.tensor.matmul(ssP[:], ones2[:], SQ[:, s],
                                         start=True, stop=True)
                        nc.vector.tensor_copy(ss_sb[:, s], ssP[:])

                    # per-half rsqrt: reshape via DRAM, recip DVE, sqrt ACT
                    # rq = sqrt(1/ssq); t*rk = sqrt(t^2/ssk) via tscaleh
                    nc.sync.dma_start(ssd[hh][0:2, :], ss_sb[:, s2])
                    sscol = chn.tile([64, 64], F16, tag="sscol")
                    nc.sync.dma_start(sscol[:], ssd_col[hh][:])
                    tcol = chn.tile([64, 64], F32, tag="tcol")
                    nc.vector.reciprocal(tcol[:], sscol[:])
                    rqcol = chn.tile([64, 64], F16, tag="rqcol")
                    nc.scalar.activation(rqcol[:], tcol[:], AF.Sqrt,
                                         scale=tscaleh[:])
                    nc.sync.dma_start(rqd_col[hh][:], rqcol[:])
                    nc.sync.dma_start(rq0[0:1, s2], rqd[hh][0:1, :])
                    nc.sync.dma_start(rk0[0:1, s2], rqd[hh][1:2, :])
                    nc.gpsimd.partition_broadcast(rqb[:, s2], rq0[0:1, s2])
                    nc.gpsimd.partition_broadcast(rkb[:, s2], rk0[0:1, s2])
                    for n in ns:
                        s = slice(512 * n, 512 * (n + 1))
                        nc.vector.tensor_mul(QHD[0:48, s], PK[0:48, s],
                                             rqb[0:48, s])
                        nc.vector.tensor_mul(KHb[64:112, s], PK[64:112, s],
                                             rkb[64:112, s])
                    nc.sync.dma_start(QHD[64:112, s2], QHD[0:48, s2])
                    nc.sync.dma_start(KHb[0:48, s2], KHb[64:112, s2])

                # vT transposes, bunched in the B tail (transpose-mode
                # is not PE-busy for HAM; keep out of the MM flow)
                for j in range(32):
                    vtP = pAB.tile([128, 48], F16, tag="B")
                    nc.tensor.transpose(vtP[:],
                                        v_sb[:, 128 * j:128 * (j + 1)],
                                        identh[0:48, 0:48])
                    nc.vector.tensor_copy(vT[:, 64 * j:64 * j + 48], vtP[:])
                # re-warm burst before attention (transposes cooled the gate)
                for r in range(28):
                    dmy = pAB.tile([64, 512], F32, tag="A")
                    nc.tensor.matmul(
                        dmy[:], vT[:, 64 * 31:64 * 31 + 64],
                        x_a[:, 512 * (r % 8):512 * (r % 8) + 512],
                        start=True, stop=True, skip_group_check=True)

            # ---- phase 7: attention
            with (
                tc.tile_pool(name="psS", bufs=3, space="PSUM") as psS,
                tc.tile_pool(name="psAV", bufs=1, space="PSUM") as psAV,
            ):
                def emit_av(avP, j, Ea, Eb):
                    first, last = j == 0, j == KCH // 2 - 1
                    for nn2 in range(2):
                        ns2 = slice(512 * nn2, 512 * (nn2 + 1))
                        nc.tensor.matmul(
                            avP[0:64, ns2], vT[:, 128 * j:128 * j + 64],
                            Ea[:, ns2], start=first, stop=last,
                            skip_group_check=True)
                        nc.tensor.matmul(
                            avP[64:128, ns2], vT[:, 128 * j + 64:128 * j + 128],
                            Eb[:, ns2], start=first, stop=last,
                            skip_group_check=True)

                def chain_dma(pq):
                    # normalization chain, DMA/DVE/PE-broadcast (latency
                    # hidden inside the following quarter)
                    p0 = QW * pq
                    qr = slice(p0, p0 + QW)
                    za = chn.tile([1, QW], F16, tag="za")
                    zb = chn.tile([1, QW], F16, tag="zb")
                    nc.sync.dma_start(za[:], U2[48:49, qr])
                    nc.sync.dma_start(zb[:], U2[112:113, qr])
                    yield
                    nc.vector.tensor_add(z_row[0:1, qr], za[:], zb[:])
                    yield
                    nc.sync.dma_start(zdr[0:1, qr], z_row[0:1, qr])
                    yield
                    zT = chn.tile([128, 8], F16, tag="zT")
                    nc.sync.dma_start(zT[:], zdr_col[:, 8 * pq:8 * pq + 8])
                    yield
                    rzc = chn.tile([128, 8], F32, tag="rzc")
                    nc.vector.reciprocal(rzc[:], zT[:])
                    rzc16 = chn.tile([128, 8], F16, tag="rzc16")
                    nc.vector.tensor_copy(rzc16[:], rzc[:])
                    yield
                    nc.sync.dma_start(rdr_col[:, 8 * pq:8 * pq + 8], rzc16[:])
                    yield
                    nc.sync.dma_start(rz_row[0:1, qr], rdr[0:1, qr])
                    yield
                    rzb = chn.tile([128, QW], F16, tag="rzb")
                    nc.gpsimd.partition_broadcast(rzb[:], rz_row[0:1, qr])
                    yield
                    nc.vector.tensor_mul(att[:, qr], U2[:, qr], rzb[:])

                def chain_pe(pq):
                    # low-latency PE/PSUM chain for the final quarter
                    p0 = QW * pq
                    for g in range(2):
                        sl = slice(p0 + 512 * g, p0 + 512 * (g + 1))
                        zP = psS.tile([1, 512], F32, tag="S")
                        nc.tensor.matmul(zP[:], zmask[:], U2[:, sl],
                                         start=True, stop=True)
                        nc.vector.tensor_copy(zh_row[0:1,
                                                     512 * g:512 * (g + 1)],
                                              zP[:])
                    zT = psS.tile([128, 16], F16, tag="S")
                    for jj in range(8):
                        nc.tensor.transpose(
                            zT[:, 2 * jj:2 * jj + 1],
                            zh_row[0:1, 128 * jj:128 * (jj + 1)],
                            identh[0:1, 0:1])
                    zT3 = zT[:].rearrange("p (j t) -> p j t", t=2)
                    rzc = chn.tile([128, 8], F32, tag="rzc")
                    nc.vector.reciprocal(rzc[:], zT3[:, :, 0])
                    nc.vector.tensor_copy(rzh[:], rzc[:])
                    for g in range(2):
                        rzP = psS.tile([1, 512], F16, tag="S")
                        for jj in range(4):
                            nc.tensor.transpose(
                                rzP[0:1, 128 * jj:128 * (jj + 1)],
                                rzh[:, 4 * g + jj:4 * g + jj + 1], identh[:])
                        nc.vector.tensor_copy(
                            rzh_row[0:1, 512 * g:512 * (g + 1)], rzP[:])
                    for g in range(2):
                        sl = slice(p0 + 512 * g, p0 + 512 * (g + 1))
                        rzbP = psS.tile([128, 512], F32, tag="S")
                        nc.tensor.matmul(
                            rzbP[:], ones_row[:],
                            rzh_row[0:1, 512 * g:512 * (g + 1)],
                            start=True, stop=True)
                        nc.vector.tensor_mul(att[:, sl], U2[:, sl], rzbP[:])

                def emit_proj(n):
                    s = slice(512 * n, 512 * (n + 1))
                    oP = psS.tile([128, 512], F32, tag="S")
                    oP2 = psS.tile([64, 512], F32, tag="S")
                    nc.tensor.matmul(oP[:], pw2a[:], att[:, s],
                                     start=True, stop=True)
                    nc.tensor.matmul(oP2[:], pw2b[:, 0:64], att[:, s],
                                     start=True, stop=True)
                    out_sb = chn.tile([128, 512], F32, tag="osb")
                    out_sb2 = chn.tile([64, 512], F32, tag="osb2")
                    nc.vector.tensor_copy(out_sb[:], oP[:])
                    nc.vector.tensor_copy(out_sb2[:], oP2[:])
                    nc.sync.dma_start(out_d[0:128, s], out_sb[:])
                    nc.sync.dma_start(out_d[128:192, s], out_sb2[:])

                chain = None
                proj_ready = []
                proj_next = []
                for qq in range(QQ):
                    q0 = QW * qq
                    avP = psAV.tile([128, QW], F32, tag="av")
                    pend = []
                    for i in range(KCH // 2):
                        ka, kb = 2 * i, 2 * i + 1
                        Sa = psS.tile([128, QW], F32, tag="S")
                        Sb = psS.tile([128, QW], F32, tag="S")
                        for nn in range(2):
                            qs = slice(q0 + 512 * nn, q0 + 512 * (nn + 1))
                            ns2 = slice(512 * nn, 512 * (nn + 1))
                            nc.tensor.matmul(
                                Sa[:, ns2],
                                KHb[0:48, 128 * ka:128 * (ka + 1)],
                                QHD[0:48, qs], start=True, stop=True)
                            nc.tensor.matmul(
                                Sb[:, ns2],
                                KHb[64:112, 128 * kb:128 * (kb + 1)],
                                QHD[64:112, qs], start=True, stop=True)
                        Ea = ep.tile([128, QW], F16, tag="E")
                        Eb = ep.tile([128, QW], F16, tag="E")
                        nc.scalar.activation(Ea[:], Sa[:], AF.Exp)
                        nc.scalar.activation(Eb[:], Sb[:], AF.Exp)
                        pend.append((Ea, Eb))
                        if i > 0:
                            Pa, Pb = pend.pop(0)
                            emit_av(avP, i - 1, Pa, Pb)
                        if chain is not None:
                            if next(chain, StopIteration) is StopIteration:
                                chain = None
                                proj_next.extend(
                                    [2 * (qq - 1), 2 * (qq - 1) + 1])
                        if i in (2, 5, 8) and proj_ready:
                            emit_proj(proj_ready.pop(0))
                    Pa, Pb = pend.pop(0)
                    emit_av(avP, KCH // 2 - 1, Pa, Pb)
                    while chain is not None:
                        if next(chain, StopIteration) is StopIteration:
                            chain = None
                            proj_next.extend([2 * (qq - 1), 2 * (qq - 1) + 1])
                    nc.vector.tensor_copy(U2[:, q0:q0 + QW], avP[:])
                    if qq < QQ - 1:
                        chain = chain_dma(qq)
                    proj_ready.extend(proj_next)
                    proj_next = []

                chain_pe(QQ - 1)
                proj_ready.extend(proj_next)
                proj_ready.extend([2 * (QQ - 1), 2 * (QQ - 1) + 1])
                for n in proj_ready:
                    emit_proj(n)

    nc.compile()
    return nc


def _get_nc():
    if "nc" not in _cache:
        _cache["nc"] = _build()
    return _cache["nc"]


def _prep_core(x, qkv_w, dw_w, proj_w, temperature, b, h):
    w1 = qkv_w[:, :, 0, 0]  # [576, 192]
    dw = dw_w[:, 0]  # [576, 3, 3]
    pwf = proj_w[:, :, 0, 0]  # [192, 192]
    qs, ks, vs = h * C, DIM + h * C, 2 * DIM + h * C
    w1qk = np.zeros((192, 112), np.float32)
    w1qk[:, 0:48] = w1[qs:qs + C].T
    w1qk[:, 64:112] = w1[ks:ks + C].T
    w1v = np.ascontiguousarray(w1[vs:vs + C].T)  # [192, 48]
    dq, dk, dv = dw[qs:qs + C], dw[ks:ks + C], dw[vs:vs + C]
    dwqk = np.zeros((112, 9, 112), np.float32)
    dwv = np.zeros((48, 9, 48), np.float32)
    ar = np.arange(C)
    for t in range(9):
        dy, dx = t // 3 - 1, t % 3 - 1
        dwqk[ar, t, ar] = dq[:, dy + 1, dx + 1]
        dwqk[64 + ar, t, 64 + ar] = dk[:, dy + 1, dx + 1]
        dwv[ar, t, ar] = dv[:, dy + 1, dx + 1]
    pw_sel = pwf[:, h * C:(h + 1) * C].T  # [48, 192]
    pw2a = np.zeros((128, 128), np.float32)
    pw2a[0:48, :] = pw_sel[:, 0:128]
    pw2a[64:112, :] = pw_sel[:, 0:128]
    pw2b = np.zeros((128, 64), np.float32)
    pw2b[0:48, :] = pw_sel[:, 128:192]
    pw2b[64:112, :] = pw_sel[:, 128:192]
    ones2 = np.zeros((112, 2), np.float32)
    ones2[0:48, 0] = 1.0
    ones2[64:112, 1] = 1.0
    tval = float(temperature[h, 0, 0])
    tscaleh = np.ones((64, 1), np.float32)
    tscaleh[32:64, 0] = tval * tval
    zmask = np.zeros((128, 1), np.float32)
    zmask[48, 0] = 1.0
    zmask[112, 0] = 1.0
    xf = x[b].reshape(DIM, HW)
    return {
        "x_a": np.ascontiguousarray(xf[0:128]).astype(np.float16),
        "x_b": np.ascontiguousarray(xf[128:192]).astype(np.float16),
        "w1qk_a": w1qk[0:128].astype(np.float16),
        "w1qk_b": w1qk[128:192].astype(np.float16),
        "w1v_a": w1v[0:128].astype(np.float16),
        "w1v_b": w1v[128:192].astype(np.float16),
        "dwqk": dwqk.reshape(112, 9 * 112).astype(np.float16),
        "dwv": dwv.reshape(48, 9 * 48).astype(np.float16),
        "pw2a": pw2a.astype(np.float16),
        "pw2b": pw2b.astype(np.float16),
        "ones2": ones2.astype(np.float16),
        "identh": np.eye(128, dtype=np.float16),
        "tscaleh": tscaleh,
        "zmask": zmask.astype(np.float16),
    }


def kernel(x, qkv_w, dw_w, proj_w, temperature):
    from concourse.bass_utils import run_bass_kernel_spmd

    nc = _get_nc()
    x = np.asarray(x, np.float32)
    qkv_w = np.asarray(qkv_w, np.float32)
    dw_w = np.asarray(dw_w, np.float32)
    proj_w = np.asarray(proj_w, np.float32)
    temperature = np.asarray(temperature, np.float32)
    in_maps = [
        _prep_core(x, qkv_w, dw_w, proj_w, temperature, c // HEADS, c % HEADS)
        for c in range(NCORES)
    ]
    res = run_bass_kernel_spmd(nc, in_maps, core_ids=list(range(NCORES)))
    out = np.zeros((B, DIM, HW), np.float32)
    for c in range(NCORES):
        out[c // HEADS] += res.results[c]["out"]
    return out.reshape(B, DIM, H, W)
